# revision 23
# baseline (speedup 1.0000x reference)
"""Trainium2 Bass kernel for nn_Attention_49177375539262 (sparse_attention).

Math (per group g of b*h*B = 512 groups, L=256, D=64):
  sigma_q = q @ Wq^T + 1        [L]
  sigma_k = k @ Wk^T + 1        [L]
  sim     = q @ k^T             [L, L]
  sim2    = sim * outer(sigma_q, sigma_k)
  theta_i = (sim * (1-I)) @ W1 = q @ (k^T @ W1) - (q_i . k_i) * W1_i
  th_g    = W2b @ leakyrelu(W2a @ theta, 0.1)      (scalar)
  attn    = softmax(sim2, -1) * (sim2 > th_g)

Sharding: data-parallel over groups; 8 cores x 64 groups each.

Device strategy per core:
  - load q,k natural [128, 128] tiles (partition p holds rows 2p, 2p+1)
  - sigma/diag via DVE/GPSIMD scalar_tensor_tensor with accum
  - scale q by sigma_q, k by sigma_k (fused (x*sig_raw)+x )
  - PE transposes -> qT [qT_even; qT_odd], kT_A (same), kT_B (swapped)
  - 4 matmuls (row-tiled pairs, K=64) -> PSUM sim2 [128, 256] per i-parity
  - rowmax (DVE reduce, negate) -> ACT exp(x - m) with sum accum -> p
  - mask+norm: (psum > theta)*recip_s via dual-op tensor_scalar on PSUM
  - attn = p * mask_scaled ; DMA out with row-interleave unpermute
"""

import sys

for _p in ("/opt/trn_rl_repo", "/opt/pypackages"):
    if _p not in sys.path:
        sys.path.append(_p)

import numpy as np

import concourse.bass as bass
import concourse.mybir as mybir
from concourse.tile import TileContext
from concourse.bass_utils import run_bass_kernel_spmd

F32 = mybir.dt.float32
BF16 = mybir.dt.bfloat16

N_CORES = 8
G = 64          # groups per core
L = 256
D = 64
P = 128

_CACHE = {}


def _build_program():
    nc = bass.Bass()

    # ---- I/O ----
    q_in = nc.declare_dram_parameter("q", [G, P, 2 * D], F32, isOutput=False)
    k_in = nc.declare_dram_parameter("k", [G, P, 2 * D], F32, isOutput=False)
    wq_b = nc.declare_dram_parameter("wq_b", [P, D], F32, isOutput=False)
    wk_b = nc.declare_dram_parameter("wk_b", [P, D], F32, isOutput=False)
    w1c = nc.declare_dram_parameter("w1c", [P, 2], F32, isOutput=False)
    w1re = nc.declare_dram_parameter("w1re", [P, P], F32, isOutput=False)
    w1ro = nc.declare_dram_parameter("w1ro", [P, P], F32, isOutput=False)
    w2aT_e = nc.declare_dram_parameter("w2aT_e", [P, 2 * P], F32, isOutput=False)
    w2aT_o = nc.declare_dram_parameter("w2aT_o", [P, 2 * P], F32, isOutput=False)
    w2bc = nc.declare_dram_parameter("w2bc", [P, 2], F32, isOutput=False)
    ident = nc.declare_dram_parameter("ident", [P, P], F32, isOutput=False)
    ones_row = nc.declare_dram_parameter("ones_row", [1, P], F32, isOutput=False)
    out = nc.declare_dram_parameter("attn", [G, P, 2, L], F32, isOutput=True)

    with TileContext(nc) as tc:
        with (
            tc.tile_pool(name="const", bufs=1) as constp,
            tc.tile_pool(name="persist", bufs=1) as persist,
            tc.tile_pool(name="nat", bufs=3) as natp,
            tc.tile_pool(name="scaled", bufs=3) as scaledp,
            tc.tile_pool(name="scratch", bufs=4) as scrp,
            tc.tile_pool(name="tsb", bufs=1) as tsbp,
            tc.tile_pool(name="soft", bufs=3) as softp,
            tc.tile_pool(name="outp", bufs=4) as outp,
            tc.tile_pool(name="ptr", bufs=3, space="PSUM") as ptr,
            tc.tile_pool(name="psim", bufs=3, space="PSUM") as psim,
            tc.tile_pool(name="psm", bufs=2, space="PSUM") as psm,
        ):
            # ---- constants to SBUF ----
            c_wq = constp.tile([P, D], F32, tag="wq")
            nc.sync.dma_start(out=c_wq, in_=wq_b[:, :])
            c_wk = constp.tile([P, D], F32, tag="wk")
            nc.sync.dma_start(out=c_wk, in_=wk_b[:, :])
            c_w1 = constp.tile([P, 2], F32, tag="w1")
            nc.sync.dma_start(out=c_w1, in_=w1c[:, :])
            c_w1re = constp.tile([P, P], F32, tag="w1re")
            nc.sync.dma_start(out=c_w1re, in_=w1re[:, :])
            c_w1ro = constp.tile([P, P], F32, tag="w1ro")
            nc.sync.dma_start(out=c_w1ro, in_=w1ro[:, :])
            c_w2ae = constp.tile([P, 2 * P], F32, tag="w2ae")
            nc.sync.dma_start(out=c_w2ae, in_=w2aT_e[:, :])
            c_w2ao = constp.tile([P, 2 * P], F32, tag="w2ao")
            nc.sync.dma_start(out=c_w2ao, in_=w2aT_o[:, :])
            c_w2b = constp.tile([P, 2], F32, tag="w2b")
            nc.sync.dma_start(out=c_w2b, in_=w2bc[:, :])
            c_id = constp.tile([P, P], F32, tag="ident")
            nc.sync.dma_start(out=c_id, in_=ident[:, :])
            c_ones = constp.tile([1, P], F32, tag="ones")
            nc.sync.dma_start(out=c_ones, in_=ones_row[:, :])

            # pre-touch consts on DVE so later fused ops need <=1 wait
            warm = scrp.tile([P, 2], F32, tag="warm")
            nc.vector.tensor_copy(warm[:, 0:1], c_wq[:, 0:1])
            nc.vector.tensor_copy(warm[:, 1:2], c_wk[:, 0:1])
            # pre-touch weight consts on PE (chained, one new dep per matmul)
            pdum = psm.tile([P, D], F32, tag="smalls")
            for cst in (c_id, c_w1re, c_w1ro, c_w2ae, c_w2ao, c_w2b):
                nc.tensor.matmul(
                    pdum[0:1, 0:1], cst[:, 0:1], c_id[:, 0:1],
                    start=True, stop=True, skip_group_check=True,
                )
            nc.tensor.matmul(
                pdum[0:1, 0:1], c_ones[:, 0:1], c_ones[:, 0:1],
                start=True, stop=True, skip_group_check=True,
            )

            # ---- persistent accumulators ----
            sq_all = persist.tile([P, 2 * G], F32, tag="sq_all")    # sigma_q raw per (c,g)
            qw1_all = persist.tile([P, 2 * G], F32, tag="qw1_all")  # (qs @ w1k) per (c,g)
            qk_all = persist.tile([P, 2 * G], F32, tag="qk_all")    # q_i.k_i per (c,g)

            # persistent transposed tensors for phase 3
            qT_all = persist.tile([P, G * P], F32, tag="qT_all")
            kTA_all = persist.tile([P, G * P], F32, tag="kTA_all")
            kTB_all = persist.tile([P, G * P], F32, tag="kTB_all")

            # ---------------- phase 1 ----------------
            for gp in range(G // 2):
              q_nat2 = natp.tile([P, 2 * 2 * D], F32, tag="q_nat")
              nc.sync.dma_start(
                  out=q_nat2.rearrange("p (g f) -> p g f", g=2),
                  in_=q_in[2 * gp : 2 * gp + 2].rearrange("g p f -> p g f"),
              )
              k_nat2 = natp.tile([P, 2 * 2 * D], F32, tag="k_nat")
              nc.sync.dma_start(
                  out=k_nat2.rearrange("p (g f) -> p g f", g=2),
                  in_=k_in[2 * gp : 2 * gp + 2].rearrange("g p f -> p g f"),
              )
              for gg in range(2):
                g = 2 * gp + gg
                q_nat = q_nat2[:, gg * 2 * D : (gg + 1) * 2 * D]
                k_nat = k_nat2[:, gg * 2 * D : (gg + 1) * 2 * D]

                # sigma_q raw (gpsimd), sigma_k raw (vector), diag q.k (gpsimd)
                sk_col = scrp.tile([P, 2], F32, tag="sk_col")
                for c in range(2):
                    sl = slice(c * D, (c + 1) * D)
                    scr = scrp.tile([P, D], F32, tag="sig_scr")
                    nc.vector.scalar_tensor_tensor(
                        out=scr, in0=q_nat[:, sl], scalar=1.0, in1=c_wq,
                        op0=mybir.AluOpType.mult, op1=mybir.AluOpType.mult,
                        accum_out=sq_all[:, 2 * g + c : 2 * g + c + 1],
                    )
                    scr2 = scrp.tile([P, D], F32, tag="sig_scr2")
                    nc.vector.scalar_tensor_tensor(
                        out=scr2, in0=k_nat[:, sl], scalar=1.0, in1=c_wk,
                        op0=mybir.AluOpType.mult, op1=mybir.AluOpType.mult,
                        accum_out=sk_col[:, c : c + 1],
                    )
                    scr3 = scrp.tile([P, D], F32, tag="dia_scr")
                    nc.vector.scalar_tensor_tensor(
                        out=scr3, in0=q_nat[:, sl], scalar=1.0, in1=k_nat[:, sl],
                        op0=mybir.AluOpType.mult, op1=mybir.AluOpType.mult,
                        accum_out=qk_all[:, 2 * g + c : 2 * g + c + 1],
                    )

                # w1k broadcast to all partitions: [128, 64] psum, base 0
                # w1k_bc[p, d] = sum_j W1_j k[j, d]
                w1k_ps = psm.tile([P, D], F32, tag="smalls")
                # PE-touch of fresh k_nat so the real matmuls carry <=1 new sem
                nc.tensor.matmul(
                    w1k_ps[0:1, 0:1], k_nat[:, 0:1], k_nat[:, 0:1],
                    start=True, stop=True, skip_group_check=True,
                )
                for c in range(2):
                    sl = slice(c * D, (c + 1) * D)
                    w1r = c_w1re if c == 0 else c_w1ro
                    nc.tensor.matmul(
                        w1k_ps, w1r, k_nat[:, sl],
                        start=(c == 0), stop=(c == 1),
                    )
                # qw1 = sum_d q_id * w1k_d  (raw q), straight into qw1_all
                for c in range(2):
                    sl = slice(c * D, (c + 1) * D)
                    scr4 = scrp.tile([P, D], F32, tag="qw1_scr")
                    nc.vector.scalar_tensor_tensor(
                        out=scr4, in0=q_nat[:, sl], scalar=1.0, in1=w1k_ps,
                        op0=mybir.AluOpType.mult, op1=mybir.AluOpType.mult,
                        accum_out=qw1_all[:, 2 * g + c : 2 * g + c + 1],
                    )

                # scale: qs = q*(sq_raw) + q ; ks likewise; ks_swap = swapped halves
                qs_nat = scaledp.tile([P, 2 * D], F32, tag="qs_nat")
                ks_nat = scaledp.tile([P, 2 * D], F32, tag="ks_nat")
                ks_swap = scaledp.tile([P, 2 * D], F32, tag="ks_swap")
                for c in range(2):
                    sl = slice(c * D, (c + 1) * D)
                    nc.vector.scalar_tensor_tensor(
                        out=qs_nat[:, sl], in0=q_nat[:, sl],
                        scalar=sq_all[:, 2 * g + c : 2 * g + c + 1], in1=q_nat[:, sl],
                        op0=mybir.AluOpType.mult, op1=mybir.AluOpType.add,
                    )
                    nc.vector.scalar_tensor_tensor(
                        out=ks_nat[:, sl], in0=k_nat[:, sl],
                        scalar=sk_col[:, c : c + 1], in1=k_nat[:, sl],
                        op0=mybir.AluOpType.mult, op1=mybir.AluOpType.add,
                    )
                    swp = slice((1 - c) * D, (2 - c) * D)
                    nc.vector.tensor_copy(ks_swap[:, swp], ks_nat[:, sl])

                # transposes (all full [128,128], PSUM base 0)
                qT_ps = ptr.tile([P, P], F32, tag="tr")
                nc.tensor.transpose(qT_ps, qs_nat, c_id)
                qT_sb = qT_all[:, g * P : (g + 1) * P]
                nc.scalar.copy(qT_sb, qT_ps)

                kTA_ps = ptr.tile([P, P], F32, tag="tr")
                nc.tensor.transpose(kTA_ps, ks_nat, c_id)
                kTA_sb = kTA_all[:, g * P : (g + 1) * P]
                nc.vector.tensor_copy(kTA_sb, kTA_ps)

                kTB_ps = ptr.tile([P, P], F32, tag="tr")
                nc.tensor.transpose(kTB_ps, ks_swap, c_id)
                kTB_sb = kTB_all[:, g * P : (g + 1) * P]
                nc.vector.tensor_copy(kTB_sb, kTB_ps)

            # ---------------- phase 2: theta + MLP ----------------
            # theta = qw1 - qk*W1
            theta_all = persist.tile([P, 2 * G], F32, tag="theta_all")
            tmp_qkw = persist.tile([P, 2 * G], F32, tag="tmp_qkw")
            # qk * W1 (W1 per (c) column, per-partition): view as [P, 2, G]
            for c in range(2):
                nc.vector.tensor_scalar(
                    out=tmp_qkw.rearrange("p (g c) -> p c g", c=2)[:, c],
                    in0=qk_all.rearrange("p (g c) -> p c g", c=2)[:, c],
                    scalar1=c_w1[:, c : c + 1], scalar2=None,
                    op0=mybir.AluOpType.mult,
                )
            nc.vector.tensor_tensor(
                out=theta_all, in0=qw1_all, in1=tmp_qkw,
                op=mybir.AluOpType.subtract,
            )

            # MLP: hdn = lrelu(W2a @ theta); th = W2b @ hdn
            hdn_sb = tsbp.tile([P, 2 * G], F32, tag="hdn")
            for ob in range(2):
                h_ps = psm.tile([P, G], F32, tag="smalls")
                for c in range(2):
                    w2a = c_w2ae if c == 0 else c_w2ao
                    nc.tensor.matmul(
                        h_ps,
                        w2a[:, ob * P : (ob + 1) * P],
                        theta_all.rearrange("p (g c) -> p c g", c=2)[:, c],
                        start=(c == 0), stop=(c == 1),
                    )
                # leaky_relu(x, 0.1) = 0.9*relu(x) + 0.1*x
                rpos = tsbp.tile([P, G], F32, tag=f"rpos{ob}")
                nc.scalar.activation(
                    rpos, h_ps, mybir.ActivationFunctionType.Relu,
                )
                h01 = tsbp.tile([P, G], F32, tag=f"h01{ob}")
                nc.vector.tensor_scalar_mul(h01, h_ps, 0.1)
                nc.vector.scalar_tensor_tensor(
                    out=hdn_sb[:, ob * G : (ob + 1) * G],
                    in0=rpos, scalar=0.9, in1=h01,
                    op0=mybir.AluOpType.mult, op1=mybir.AluOpType.add,
                )
            th_ps = psm.tile([G, 1], F32, tag="smalls")
            for ob in range(2):
                nc.tensor.matmul(
                    th_ps,
                    hdn_sb[:, ob * G : (ob + 1) * G],
                    c_w2b[:, ob : ob + 1],
                    start=(ob == 0), stop=(ob == 1),
                )
            th_sb = tsbp.tile([G, 1], F32, tag="th_col")
            nc.vector.tensor_copy(th_sb, th_ps)
            thr_ps = psm.tile([1, G], F32, tag="smalls")
            nc.tensor.transpose(thr_ps, th_sb, c_id[0:G, 0:G])
            thr_sb = tsbp.tile([1, G], F32, tag="th_row")
            nc.vector.tensor_copy(thr_sb, thr_ps)
            thb_ps = psm.tile([P, G], F32, tag="smalls")
            nc.tensor.matmul(thb_ps, c_ones, thr_sb, start=True, stop=True)
            th_bc = tsbp.tile([P, G], F32, tag="th_bc")
            nc.vector.tensor_copy(th_bc, thb_ps)

            # ---------------- phase 3: sim2 + masked softmax ----------------
            for g in range(G):
                qT_sb = qT_all[:, g * P : (g + 1) * P]
                kTA_sb = kTA_all[:, g * P : (g + 1) * P]
                kTB_sb = kTB_all[:, g * P : (g + 1) * P]
                a_full = outp.tile([P, 2 * L], F32, tag="a_sb")
                for ci in range(2):  # i-parity chunk
                    sim_ps = psim.tile([P, L], F32, tag="sim")
                    lhs = qT_sb[ci * D : (ci + 1) * D, :]
                    rA = kTA_sb if ci == 0 else kTB_sb
                    rB = kTB_sb if ci == 0 else kTA_sb
                    # cols 0:128 <- same-parity j block, 128:256 <- other
                    nc.tensor.matmul(
                        sim_ps[:, 0:P], lhs, rA[ci * D : (ci + 1) * D, :],
                        start=True, stop=True,
                        tile_position=(ci * D, 0),
                    )
                    nc.tensor.matmul(
                        sim_ps[:, P : 2 * P], lhs, rB[ci * D : (ci + 1) * D, :],
                        start=True, stop=True,
                        tile_position=(ci * D, 0),
                    )
                    # rowmax -> negated
                    neg_m = softp.tile([P, 1], F32, tag="neg_m")
                    nc.vector.tensor_reduce(
                        neg_m, sim_ps, axis=mybir.AxisListType.X,
                        op=mybir.AluOpType.max, negate=True,
                    )
                    # p = exp(sim2 - m), s = rowsum
                    s_col = softp.tile([P, 1], F32, tag="s_col")
                    p_sb = softp.tile([P, L], F32, tag="p_sb")
                    # write with j-interleave: psum cols (c j) -> true j order
                    p_wr = p_sb[:, :].rearrange("p (j c) -> p c j", c=2)
                    sim_rd = sim_ps[:, :].rearrange("p (c j) -> p c j", c=2)
                    nc.scalar.activation(
                        p_wr, sim_rd, mybir.ActivationFunctionType.Exp,
                        bias=neg_m, scale=1.0, accum_out=s_col,
                    )
                    r_col = softp.tile([P, 1], F32, tag="r_col")
                    nc.vector.reciprocal(r_col, s_col)
                    # mask_scaled = (sim2 > theta) * recip_s   (exact fp32 compare)
                    msk = softp.tile([P, L], F32, tag="msk")
                    msk_wr = msk[:, :].rearrange("p (j c) -> p c j", c=2)
                    nc.vector.tensor_scalar(
                        out=msk_wr, in0=sim_rd,
                        scalar1=th_bc[:, g : g + 1], scalar2=r_col,
                        op0=mybir.AluOpType.is_gt, op1=mybir.AluOpType.mult,
                    )
                    # attn = p * mask_scaled
                    nc.vector.tensor_mul(
                        a_full[:, ci * L : (ci + 1) * L], p_sb, msk
                    )
                nc.sync.dma_start(
                    out=out[g].rearrange("p a b -> p (a b)"), in_=a_full
                )

    _split_multi_waits(nc)
    return nc


def _split_multi_waits(nc):
    """Compute-engine instruction structs encode a single sync-wait; move
    extra waits onto standalone NoOps in front of the instruction."""
    import dataclasses

    nid = [0]
    for f in nc.m.functions:
        for bb in f.blocks:
            new = []
            for inst in bb.instructions:
                si = inst.sync_info
                if si is not None and len(si.on_wait) > 1:
                    for w in si.on_wait[:-1]:
                        nid[0] += 1
                        nop = mybir.InstNoOp(
                            name=f"IWS-{nid[0]}",
                            engine=inst.engine,
                            ins=[],
                            outs=[],
                            sync_info=mybir.SyncInfo(on_wait=[w], on_update=[]),
                            bass_nofuse=True,
                        )
                        new.append(nop)
                    inst.sync_info = dataclasses.replace(
                        si, on_wait=[si.on_wait[-1]]
                    )
                new.append(inst)
            bb.instructions[:] = new


def _prep_consts(Wq, bq, Wk, bk, W1, W2a, W2b):
    wq_b = np.broadcast_to(Wq[0][None, :], (P, D)).astype(np.float32).copy()
    wk_b = np.broadcast_to(Wk[0][None, :], (P, D)).astype(np.float32).copy()
    w1c = np.stack([W1[0, 0::2], W1[0, 1::2]], axis=1).astype(np.float32).copy()
    w1re = np.broadcast_to(W1[0, 0::2][:, None], (P, P)).astype(np.float32).copy()
    w1ro = np.broadcast_to(W1[0, 1::2][:, None], (P, P)).astype(np.float32).copy()
    w2aT = np.ascontiguousarray(W2a.T)  # [i, o]
    w2aT_e = np.ascontiguousarray(w2aT[0::2, :])
    w2aT_o = np.ascontiguousarray(w2aT[1::2, :])
    w2bc = np.stack([W2b[0, 0:P], W2b[0, P : 2 * P]], axis=1).astype(np.float32).copy()
    ident = np.eye(P, dtype=np.float32)
    ones_row = np.ones((1, P), dtype=np.float32)
    return dict(
        wq_b=wq_b, wk_b=wk_b, w1c=w1c, w1re=w1re, w1ro=w1ro,
        w2aT_e=w2aT_e, w2aT_o=w2aT_o,
        w2bc=w2bc, ident=ident, ones_row=ones_row,
    )


def kernel(q, k, Wq, bq, Wk, bk, W1, W2a, W2b, _trace=False, _trace_kwargs=None):
    q = np.ascontiguousarray(np.asarray(q, dtype=np.float32))
    k = np.ascontiguousarray(np.asarray(k, dtype=np.float32))
    b, h, B, Lq, Dq = q.shape
    GT = b * h * B
    qf = q.reshape(GT, Lq, Dq)
    kf = k.reshape(GT, Lq, Dq)

    consts = _prep_consts(
        np.asarray(Wq), np.asarray(bq), np.asarray(Wk), np.asarray(bk),
        np.asarray(W1), np.asarray(W2a), np.asarray(W2b),
    )

    if "nc" not in _CACHE:
        _CACHE["nc"] = _build_program()
    nc = _CACHE["nc"]

    gpc = GT // N_CORES
    in_maps = []
    for c in range(N_CORES):
        qs = qf[c * gpc : (c + 1) * gpc].reshape(gpc, P, 2 * Dq)
        ks = kf[c * gpc : (c + 1) * gpc].reshape(gpc, P, 2 * Dq)
        m = {"q": np.ascontiguousarray(qs), "k": np.ascontiguousarray(ks)}
        m.update(consts)
        in_maps.append(m)

    kwargs = {}
    if _trace:
        kwargs["trace"] = True
        if _trace_kwargs:
            kwargs.update(_trace_kwargs)
    res = run_bass_kernel_spmd(nc, in_maps, core_ids=list(range(N_CORES)), **kwargs)
    outs = [r["attn"].reshape(gpc, Lq, Lq) for r in res.results]
    full = np.concatenate(outs, axis=0).reshape(b, h, B, Lq, Lq)
    if _trace:
        _CACHE["last_result"] = res
    return full
